# revision 1
# baseline (speedup 1.0000x reference)
"""MAB (pre-norm multihead attention block) Trainium2 kernel.

Data-parallel over batch: B=8 batch elements -> 8 NeuronCores, no collectives.
Each core runs the full MAB for one batch element:
    Qn = LN(Q); Kn = LN(K)
    Qp = Qn@Wq.T+bq ; Kp = Kn@Wk.T+bk ; Vp = Kn@Wv.T+bv   (16 heads x 64)
    A  = double-masked softmax(Qp Kp^T / 32)
    O  = Q + A@Vp ; On = LN(O)
    O2 = O + relu(On@Wo.T+bo) ; out = LN(O2)*g_f + be_f

Host-side prep folds LN gains/biases into the weights:
    W*_eff = W* x g_ln[None,:]  (shipped pre-transposed, [D_in, D_out] layout)
    b*_eff = b* + W* @ be_ln
"""

import os
from contextlib import ExitStack

import numpy as np

import concourse.bass as bass
import concourse.tile as tile
from concourse import bacc, mybir
from concourse.masks import make_identity

F32 = mybir.dt.float32
F32R = mybir.dt.float32r
BF16 = mybir.dt.bfloat16
AF = mybir.ActivationFunctionType
ALU = mybir.AluOpType

P = 128
S = 1024          # sequence length (SQ == SK)
D = 1024          # model dim
H = 16            # heads
DH = 64           # head dim
NT = S // P       # 8 row tiles
QB = 512          # matmul moving-block (PSUM bank = 512 fp32)
EPS = 1e-5
SCALE = 1.0 / 32.0  # 1/sqrt(D)
NCORES = 8

E_BUFS = 12       # bf16 [128,1024] attention-prob tiles in flight


def _ln_tile(nc, pool, x_ap, out_ap, eps_col):
    """LayerNorm (no affine) of a [128, 1024] fp32 SBUF tile along free dim."""
    stats = pool.tile([P, 2, 6], F32, tag="stats", name="stats")
    nc.vector.bn_stats(out=stats[:, 0, :], in_=x_ap[:, 0:512])
    nc.vector.bn_stats(out=stats[:, 1, :], in_=x_ap[:, 512:1024])
    mv = pool.tile([P, 2], F32, tag="mv", name="mv")
    nc.vector.bn_aggr(out=mv, in_=stats)
    sd = pool.tile([P, 1], F32, tag="sd", name="sd")
    nc.scalar.activation(out=sd, in_=mv[:, 1:2], func=AF.Sqrt, bias=eps_col)
    rstd = pool.tile([P, 1], F32, tag="rstd", name="rstd")
    nc.vector.reciprocal(out=rstd, in_=sd)
    nc.vector.tensor_scalar(
        out=out_ap, in0=x_ap,
        scalar1=mv[:, 0:1], scalar2=rstd,
        op0=ALU.subtract, op1=ALU.mult,
    )


def _build_nc():
    nc = bacc.Bacc("TRN2", target_bir_lowering=False, debug=False)

    q_h = nc.declare_dram_parameter("q", [S, D], F32, isOutput=False)
    k_h = nc.declare_dram_parameter("k", [S, D], F32, isOutput=False)
    mask_h = nc.declare_dram_parameter("mask", [S], F32, isOutput=False)
    wqT_h = nc.declare_dram_parameter("wqT", [D, D], F32R, isOutput=False)
    wkT_h = nc.declare_dram_parameter("wkT", [D, D], F32R, isOutput=False)
    wvT_h = nc.declare_dram_parameter("wvT", [D, D], F32R, isOutput=False)
    woT_h = nc.declare_dram_parameter("woT", [D, D], F32R, isOutput=False)
    biases_h = nc.declare_dram_parameter("biases", [5, D], F32R, isOutput=False)
    gf_h = nc.declare_dram_parameter("gf", [D], F32, isOutput=False)
    bf_h = nc.declare_dram_parameter("bf", [D], F32, isOutput=False)
    out_h = nc.declare_dram_parameter("out", [S, D], F32, isOutput=True)

    def bcast_ap(vec_ap, parts=P):
        return bass.AP(tensor=vec_ap.tensor, offset=vec_ap.offset,
                       ap=[[0, parts]] + vec_ap.ap)

    with tile.TileContext(nc) as tc, ExitStack() as ctx:
        persist = ctx.enter_context(tc.tile_pool(name="persist", bufs=1))
        small = ctx.enter_context(tc.tile_pool(name="small", bufs=6))
        io = ctx.enter_context(tc.tile_pool(name="io", bufs=3))
        psT = ctx.enter_context(tc.tile_pool(name="psT", bufs=2, space="PSUM"))
        psMM = ctx.enter_context(tc.tile_pool(name="psMM", bufs=2, space="PSUM"))
        psAV = ctx.enter_context(tc.tile_pool(name="psAV", bufs=1, space="PSUM"))

        # ---- constants ----
        identity = persist.tile([P, P], F32)
        make_identity(nc, identity)
        eps_col = persist.tile([P, 1], F32)
        nc.vector.memset(eps_col, EPS)
        # constA: bias rows at partitions 0/32/64 (bq,bk,bv effective);
        # constA2 row 0 holds bo (base_partition is limited to {0,32,64})
        constA = persist.tile([P, D], F32R)
        for i, row in enumerate((0, 32, 64)):
            nc.sync.dma_start(out=constA[row:row + 1, :], in_=biases_h[i:i + 1, :])
        constA2 = persist.tile([P, D], F32R)
        nc.sync.dma_start(out=constA2[0:1, :], in_=biases_h[3:4, :])
        # constB: all-ones rows 0/32/64 (K=1 matmul operands; loaded via DMA
        # because memset cannot write float32r)
        constB = persist.tile([P, D], F32R)
        for row in (0, 32, 64):
            nc.sync.dma_start(out=constB[row:row + 1, :], in_=biases_h[4:5, :])
        # mask -> additive exp bias per k-tile column: (m-1)*1e4
        m_raw = small.tile([P, NT], F32)
        nc.sync.dma_start(out=m_raw, in_=mask_h[:].rearrange("(t p) -> p t", t=NT))
        mb = persist.tile([P, NT], F32)
        nc.vector.tensor_scalar(out=mb, in0=m_raw, scalar1=1.0, scalar2=10000.0,
                                op0=ALU.subtract, op1=ALU.mult)

        # ---- stages A-C share the projection buffers ----
        bc_stack = ctx.enter_context(ExitStack())
        qpt_pool = bc_stack.enter_context(tc.tile_pool(name="qpt", side="right", bufs=NT))
        kpt_pool = bc_stack.enter_context(tc.tile_pool(name="kpt", side="right", bufs=NT))
        vpa_pool = bc_stack.enter_context(tc.tile_pool(name="vpa", side="right", bufs=NT))
        qpT = [qpt_pool.tile([P, S], F32R, tag="qpt", name=f"qpT{i}")
               for i in range(NT)]
        kpT = [kpt_pool.tile([P, S], F32R, tag="kpt", name=f"kpT{i}")
               for i in range(NT)]
        vpa = [vpa_pool.tile([P, H * (DH + 1)], BF16, tag="vpa", name=f"vpa{i}")
               for i in range(NT)]

        def ln_transpose(src_h, dstT):
            """Stage A: LN a DRAM [S,D] tensor row-tile-wise, transpose to [D,S]."""
            for st in range(NT):
                x = io.tile([P, D], F32, tag="x", name="x")
                nc.sync.dma_start(out=x, in_=src_h[st * P:(st + 1) * P, :])
                _ln_tile(nc, small, x, x, eps_col)
                for dt in range(NT):
                    pt = psT.tile([P, P], F32, tag="pt", name="pt")
                    nc.tensor.transpose(pt, x[:, dt * P:(dt + 1) * P], identity)
                    nc.any.tensor_copy(
                        out=dstT[dt][:, st * P:(st + 1) * P], in_=pt)

        def proj_form1(wT_h, xnT, dst, brow, wblk):
            """out[v_tile, s] = sum_d WT[d,v].T @ xnT[d,s] + bias row brow."""
            for vt in range(NT):
                pq = psMM.tile([P, S], F32, tag="ps", name="pq")
                for dt in range(NT):
                    w = wblk.tile([P, P], F32R, tag="wb", name="w")
                    nc.sync.dma_start(
                        out=w,
                        in_=wT_h[dt * P:(dt + 1) * P, vt * P:(vt + 1) * P])
                    for qb in range(2):
                        nc.tensor.matmul(
                            pq[:, qb * QB:(qb + 1) * QB],
                            lhsT=(w),
                            rhs=(xnT[dt][:, qb * QB:(qb + 1) * QB]),
                            start=(dt == 0), stop=False)
                for qb in range(2):  # K=1 bias row
                    nc.tensor.matmul(
                        pq[:, qb * QB:(qb + 1) * QB],
                        lhsT=(constA[brow:brow + 1, vt * P:(vt + 1) * P]),
                        rhs=(constB[brow:brow + 1, qb * QB:(qb + 1) * QB]),
                        start=False, stop=True)
                nc.any.tensor_copy(out=dst[vt], in_=pq)

        with tc.tile_pool(name="knt", side="right", bufs=NT) as knt_pool:
            knT = [knt_pool.tile([P, S], F32R, tag="knt", name=f"knT{i}")
                   for i in range(NT)]
            with tc.tile_pool(name="qnt", side="right", bufs=NT) as qnt_pool:
                qnT = [qnt_pool.tile([P, S], F32R, tag="qnt", name=f"qnT{i}")
                       for i in range(NT)]
                ln_transpose(q_h, qnT)
                ln_transpose(k_h, knT)
                with tc.tile_pool(name="wblkq", side="right", bufs=3) as wblkq:
                    proj_form1(wqT_h, qnT, qpT, 0, wblkq)

            with tc.tile_pool(name="wblkk", side="right", bufs=3) as wblkk:
                proj_form1(wkT_h, knT, kpT, 32, wblkk)

            # Vp in [S, V] layout + interleaved ones column (softmax denom),
            # WvT streamed in 512-column halves to cap SBUF
            for kt in range(NT):
                nc.vector.memset(vpa[kt], 1.0)
            with tc.tile_pool(name="wvp", side="right", bufs=NT) as wvp:
                for qb in range(2):
                    wvh = [wvp.tile([P, QB], F32R, tag="wv", name=f"wv{i}")
                           for i in range(NT)]
                    for dt in range(NT):
                        nc.sync.dma_start(
                            out=wvh[dt],
                            in_=wvT_h[dt * P:(dt + 1) * P, qb * QB:(qb + 1) * QB])
                    for kt in range(NT):
                        pv = psMM.tile([P, QB], F32, tag="ps", name="pv")
                        for dt in range(NT):
                            nc.tensor.matmul(
                                pv,
                                lhsT=(knT[dt][:, kt * P:(kt + 1) * P]),
                                rhs=(wvh[dt]),
                                start=(dt == 0), stop=False)
                        nc.tensor.matmul(
                            pv,
                            lhsT=(constB[64:65, 0:P]),
                            rhs=(constA[64:65, qb * QB:(qb + 1) * QB]),
                            start=False, stop=True)
                        nc.any.tensor_copy(
                            out=vpa[kt].rearrange(
                                "p (h x) -> p h x", x=DH + 1)[:, qb * 8:(qb + 1) * 8, 0:DH],
                            in_=pv.rearrange("p (h x) -> p h x", x=DH))

        # ---- stage C: attention, per head pair (row-packed on the PE) ----
        opool = ctx.enter_context(tc.tile_pool(name="opool", bufs=NT))
        O_sb = [opool.tile([P, D], F32, tag="o", name=f"O{i}") for i in range(NT)]
        epool = bc_stack.enter_context(tc.tile_pool(name="epool", side="right", bufs=E_BUFS))
        otpool = bc_stack.enter_context(tc.tile_pool(name="otpool", side="right", bufs=2))

        for hp in range(H // 2):
            vt = hp  # QpT/KpT partition-tile holding heads 2hp (rows 0:64) and 2hp+1 (64:128)
            e_tiles = {0: [], 1: []}
            for kt in range(NT):
                sps = {}
                for par in range(2):  # head parity: rows 0:64 / 64:128
                    po = par * DH
                    ps = psMM.tile([P, S], F32, tag="ps", name="sps")
                    sps[par] = ps
                    for qb in range(2):
                        nc.tensor.matmul(
                            ps[:, qb * QB:(qb + 1) * QB],
                            lhsT=(kpT[vt][po:po + DH, kt * P:(kt + 1) * P]),
                            rhs=(qpT[vt][po:po + DH, qb * QB:(qb + 1) * QB]))
                for par in range(2):
                    e = epool.tile([P, S], BF16, tag="et", name="e")
                    nc.scalar.activation(out=e, in_=sps[par], func=AF.Exp,
                                         bias=mb[:, kt:kt + 1], scale=SCALE)
                    e_tiles[par].append(e)
            for par in range(2):
                h = 2 * hp + par
                avp = psAV.tile([DH + 1, S], F32, tag="av", name="avp")
                for kt in range(NT):
                    for qb in range(2):
                        nc.tensor.matmul(
                            avp[:, qb * QB:(qb + 1) * QB],
                            lhsT=vpa[kt][:, h * (DH + 1):(h + 1) * (DH + 1)],
                            rhs=e_tiles[par][kt][:, qb * QB:(qb + 1) * QB],
                            start=(kt == 0), stop=(kt == NT - 1))
                ot = otpool.tile([DH + 1, S], F32, tag="ot", name="ot")
                nc.any.tensor_copy(out=ot, in_=avp)
                for qt in range(NT):
                    pt = psT.tile([P, DH + 1], F32, tag="pt", name="ptv")
                    nc.tensor.transpose(
                        pt, ot[:, qt * P:(qt + 1) * P], identity[0:DH + 1, 0:DH + 1])
                    rcp = small.tile([P, 1], F32, tag="rcp", name="rcp")
                    nc.vector.reciprocal(rcp, pt[:, DH:DH + 1])
                    nc.vector.tensor_scalar_mul(
                        out=O_sb[qt][:, h * DH:(h + 1) * DH],
                        in0=pt[:, 0:DH], scalar1=rcp)

        bc_stack.close()  # free qpT/kpT/vpa/E buffers before stage D

        # ---- stage D: residual + LN + FC(relu) + residual + final LN ----
        with tc.tile_pool(name="onp", bufs=2) as onp, \
             tc.tile_pool(name="ontp", bufs=NT) as ontp, \
             tc.tile_pool(name="wop", bufs=NT) as wop, \
             tc.tile_pool(name="fin", bufs=1) as fin, \
             tc.tile_pool(name="zp", bufs=2) as zp:
            # final-LN affine, broadcast across partitions
            gf_bc = fin.tile([P, D], F32)
            nc.sync.dma_start(out=gf_bc, in_=bcast_ap(gf_h[:]))
            bf_bc = fin.tile([P, D], F32)
            nc.sync.dma_start(out=bf_bc, in_=bcast_ap(bf_h[:]))
            onT = [ontp.tile([P, S], F32R, tag="ont", name=f"onT{i}")
                   for i in range(NT)]
            for st in range(NT):
                q2 = io.tile([P, D], F32, tag="x", name="q2")
                nc.sync.dma_start(out=q2, in_=q_h[st * P:(st + 1) * P, :])
                nc.vector.tensor_add(out=O_sb[st], in0=O_sb[st], in1=q2)
                on = onp.tile([P, D], F32, tag="on", name="on")
                _ln_tile(nc, small, O_sb[st], on, eps_col)
                for dt in range(NT):
                    pt = psT.tile([P, P], F32, tag="pt", name="pto")
                    nc.tensor.transpose(pt, on[:, dt * P:(dt + 1) * P], identity)
                    nc.any.tensor_copy(out=onT[dt][:, st * P:(st + 1) * P], in_=pt)

            wo = [wop.tile([P, D], F32R, tag="wo", name=f"wo{i}") for i in range(NT)]
            for dt in range(NT):
                nc.sync.dma_start(out=wo[dt], in_=woT_h[dt * P:(dt + 1) * P, :])
            for st in range(NT):
                pz = psMM.tile([P, S], F32, tag="ps", name="pz")
                for dt in range(NT):
                    for qb in range(2):
                        nc.tensor.matmul(
                            pz[:, qb * QB:(qb + 1) * QB],
                            lhsT=(onT[dt][:, st * P:(st + 1) * P]),
                            rhs=(wo[dt][:, qb * QB:(qb + 1) * QB]),
                            start=(dt == 0), stop=False)
                for qb in range(2):
                    nc.tensor.matmul(
                        pz[:, qb * QB:(qb + 1) * QB],
                        lhsT=(constB[0:1, 0:P]),
                        rhs=(constA2[0:1, qb * QB:(qb + 1) * QB]),
                        start=False, stop=True)
                z = zp.tile([P, D], F32, tag="z", name="z")
                nc.scalar.activation(out=z, in_=pz, func=AF.Relu, bias=0.0)
                nc.vector.tensor_add(out=z, in0=z, in1=O_sb[st])
                _ln_tile(nc, small, z, z, eps_col)
                nc.vector.tensor_mul(out=z, in0=z, in1=gf_bc)
                nc.vector.tensor_add(out=z, in0=z, in1=bf_bc)
                nc.sync.dma_start(out=out_h[st * P:(st + 1) * P, :], in_=z)

    nc.compile()
    return nc


_NC = None


def _get_nc():
    global _NC
    if _NC is None:
        _NC = _build_nc()
    return _NC


def _host_prep(inputs):
    f = lambda k: np.asarray(inputs[k], np.float32)
    Q, K, pm = f("Q"), f("K"), f("pad_mask")
    Wq, Wk, Wv, Wo = f("Wq"), f("Wk"), f("Wv"), f("Wo")
    bq, bk, bv, bo = f("bq"), f("bk"), f("bv"), f("bo")
    g_q, be_q = f("g_q"), f("be_q")
    g_kv, be_kv = f("g_kv"), f("be_kv")
    g_o, be_o = f("g_o"), f("be_o")
    g_f, be_f = f("g_f"), f("be_f")

    wqT = np.ascontiguousarray((Wq * g_q[None, :]).T)
    wkT = np.ascontiguousarray((Wk * g_kv[None, :]).T)
    wvT = np.ascontiguousarray((Wv * g_kv[None, :]).T)
    woT = np.ascontiguousarray((Wo * g_o[None, :]).T)
    beff = np.stack([bq + Wq @ be_q, bk + Wk @ be_kv,
                     bv + Wv @ be_kv, bo + Wo @ be_o,
                     np.ones(D, np.float32)]).astype(np.float32)
    shared = {"wqT": wqT, "wkT": wkT, "wvT": wvT, "woT": woT,
              "biases": beff, "gf": g_f, "bf": be_f}
    in_maps = [dict(shared, q=np.ascontiguousarray(Q[i]),
                    k=np.ascontiguousarray(K[i]),
                    mask=np.ascontiguousarray(pm[i]))
               for i in range(NCORES)]
    return in_maps


LAST_RESULTS = None


def kernel(**inputs):
    from concourse.bass_utils import run_bass_kernel_spmd

    global LAST_RESULTS
    nc = _get_nc()
    in_maps = _host_prep(inputs)
    res = run_bass_kernel_spmd(nc, in_maps, core_ids=list(range(NCORES)))
    LAST_RESULTS = res
    return np.stack([res.results[i]["out"] for i in range(NCORES)]).astype(np.float32)

